# revision 18
# baseline (speedup 1.0000x reference)
"""FNO block (nn_FNOBlock_48962627175213) as a Bass/Tile kernel on 8 trn2 cores.

Math: only 64 complex rfft modes (32 low + 32 high) survive into out_ft, so
rfft/irfft collapse into skinny DFT matmuls against precomputed fp32 bases.
Data-parallel over batch: each core takes 4 of the 32 batches.

Per-core pipeline (rows = (b, c) b-major, 256 rows of length L=8192):
  1. head: phi = emb @ A^T (all four parts), FiLM MLP, per-batch scaled
     time weights (1+gamma folded into lin_w^T), folded bias vector.
  2. PE-transpose x tiles -> x^T chunks; fwd DFT: RT[modecol, row] +=
     F_chunk^T @ xT_chunk (64 accumulating matmuls).
  3. XS: per (branch, b) transpose-matmuls of RT blocks against runtime
     diagonal matrices built from phi -> folds the complex phi multiply
     into the layout shuffle (out_pos * phi == (x_ft * phi) @ w).
  4. spectral: 128 small matmuls [K=128 (re i, im i), M=64 o, N=4 b].
  5. R2 transposes -> R2f [(d,br,m), (b,o)] = inverse-DFT lhsT.
  6. inverse DFT + time branch accumulate into one PSUM tile; ACT applies
     silu(psum + folded_bias) and output DMAs stream out.
"""
import sys

if '/opt/trn_rl_repo' not in sys.path:
    sys.path.insert(0, '/opt/trn_rl_repo')

import numpy as np

import concourse.bass as bass
import concourse.mybir as mybir
from concourse.tile import TileContext
from concourse.bass_utils import run_bass_kernel_spmd

FP = mybir.dt.float32
AF = mybir.ActivationFunctionType

B, C, L, M, EMB, HID = 32, 64, 8192, 32, 256, 64
K = L // 2 + 1
NEG0 = K - M          # 4065
N_CORES = 8
B_LOC = B // N_CORES  # 4
ROWS = B_LOC * C      # 256


# --------------------------------------------------------------------------
# host-side constant builders
# --------------------------------------------------------------------------
def _build_constants(weights_pos, weights_neg, A_real_pos, A_imag_pos,
                     A_real_neg, A_imag_neg, tm_w1, tm_b1, tm_w2, tm_b2,
                     lin_w, lin_b):
    n = np.arange(L, dtype=np.float64)
    s = 1.0 / np.sqrt(L)

    # fwd DFT basis [8192, 128], col = br*64 + d*32 + m
    F = np.zeros((L, 128), np.float64)
    for br in range(2):
        for m in range(M):
            k = m if br == 0 else NEG0 + m
            ang = 2.0 * np.pi * k * n / L
            F[:, br * 64 + m] = np.cos(ang) * s
            F[:, br * 64 + 32 + m] = -np.sin(ang) * s
    F_sb = F.reshape(64, 128, 128).transpose(1, 0, 2).reshape(128, 64 * 128)
    F_sb = np.ascontiguousarray(F_sb, np.float32)

    # inverse basis [128, 8192], row = d*64 + br*32 + m (pocketfft irfft
    # semantics: Im parts of DC and Nyquist are discarded)
    G = np.zeros((128, L), np.float64)
    for br in range(2):
        for m in range(M):
            k = m if br == 0 else NEG0 + m
            ang = 2.0 * np.pi * k * n / L
            if k == 0:
                G[br * 32 + m] = s
            elif k == L // 2:
                G[br * 32 + m] = np.cos(np.pi * n) * s
            else:
                G[br * 32 + m] = 2.0 * np.cos(ang) * s
                G[64 + br * 32 + m] = -2.0 * np.sin(ang) * s
    G = np.ascontiguousarray(G, np.float32)

    # spectral weights [128, 8192]: col = ((br*32+m)*2+dout)*64 + o,
    # rows = (din, i); dout=0 -> [wr; -wi], dout=1 -> [wi; wr]
    Wspec = np.zeros((128, 8192), np.float32)
    for br, wfull in ((0, weights_pos), (1, weights_neg)):
        for m in range(M):
            wr = wfull[:, :, m, 0]
            wi = wfull[:, :, m, 1]
            c0 = ((br * 32 + m) * 2 + 0) * 64
            c1 = ((br * 32 + m) * 2 + 1) * 64
            Wspec[0:64, c0:c0 + 64] = wr
            Wspec[64:128, c0:c0 + 64] = -wi
            Wspec[0:64, c1:c1 + 64] = wi
            Wspec[64:128, c1:c1 + 64] = wr

    # phi projector [256, 128]: col = d*64 + br*32 + m; then chunk-major
    # repack to [128, 2*128] since SBUF tiles cap at 128 partitions
    Astack = np.zeros((EMB, 128), np.float32)
    Astack[:, 0:32] = A_real_pos.T
    Astack[:, 32:64] = A_real_neg.T
    Astack[:, 64:96] = A_imag_pos.T
    Astack[:, 96:128] = A_imag_neg.T
    Astack = np.ascontiguousarray(
        Astack.reshape(2, 128, 128).transpose(1, 0, 2).reshape(128, 256))

    w1T = tm_w1.T.astype(np.float32)  # [256, 64] -> [128, 2*64]
    w1T = np.ascontiguousarray(
        w1T.reshape(2, 128, 64).transpose(1, 0, 2).reshape(128, 128))

    return dict(
        F=F_sb, G=G, W=Wspec, A=Astack,
        w1T=w1T,
        b1=np.ascontiguousarray(tm_b1[:, None], np.float32),
        w2T=np.ascontiguousarray(tm_w2.T, np.float32),
        b2r=np.ascontiguousarray(np.tile(tm_b2, (4, 1)), np.float32),
        lbr=np.ascontiguousarray(np.tile(lin_b, (4, 1)), np.float32),
        lwT2=np.ascontiguousarray(np.tile(lin_w.T, (2, 1)), np.float32),
        ones=np.ones((1, 64), np.float32),
        id128=np.eye(128, dtype=np.float32),
        idstack=np.ascontiguousarray(np.tile(np.eye(32), (4, 1)), np.float32),
        nidstack=np.ascontiguousarray(np.tile(-np.eye(32), (4, 1)), np.float32),
    )


# --------------------------------------------------------------------------
# walrus workaround: this container's walrus rejects >1 sync-wait on
# TPB_CTRL lowering (Drain/NoOp). Split extra waits onto preceding NOPs.
# --------------------------------------------------------------------------
def _split_multiwait(nc, max_waits=1):
    for f in nc.m.functions:
        for blk in f.blocks:
            new = []
            changed = False
            for inst in blk.instructions:
                si = inst.sync_info
                if (si is not None and len(si.on_wait) > max_waits):
                    waits = list(si.on_wait)
                    head, tail = waits[:-max_waits], waits[-max_waits:]
                    for j, w in enumerate(head):
                        nop = mybir.InstNoOp(name=f"{inst.name}-ws{j}",
                                             ins=[], outs=[])
                        nop.engine = inst.engine
                        nop.sync_info = mybir.SyncInfo(on_wait=[w], on_update=[])
                        new.append(nop)
                    inst.sync_info = mybir.SyncInfo(on_wait=tail,
                                                    on_update=list(si.on_update))
                    changed = True
                new.append(inst)
            if changed:
                blk.instructions = new


# --------------------------------------------------------------------------
# the bass program (input-value independent; built once)
# --------------------------------------------------------------------------
def _build_nc(split=True, sim_safe=False):
    nc = bass.Bass("TRN2")
    d = {}
    for name, shape in (
        ("x4", [ROWS, L]), ("embT", [128, 2 * B_LOC]),
        ("F", [128, 8192]), ("G", [128, 8192]), ("W", [128, 8192]),
        ("A", [128, 256]),
        ("w1T", [128, 128]), ("b1", [64, 1]), ("w2T", [64, 128]),
        ("b2r", [4, 128]), ("lbr", [4, 64]), ("lwT2", [128, 64]),
        ("ones", [1, 64]), ("id128", [128, 128]),
        ("idstack", [128, 32]), ("nidstack", [128, 32]),
    ):
        d[name] = nc.dram_tensor(name, shape, FP, kind="ExternalInput")
    y = nc.dram_tensor("y", [ROWS, L], FP, kind="ExternalOutput")

    with TileContext(nc) as tc:
        from contextlib import ExitStack

        def act_silu(out_ap, in_ap, bias_ap, zscratch):
            # silu(z), z = in + bias. sim_safe path avoids the Silu LUT
            # (not implemented in CoreSim): z*sigmoid(z) via ACT+DVE.
            if not sim_safe:
                nc.scalar.activation(out_ap, in_ap, AF.Silu, bias=bias_ap)
            else:
                nc.scalar.activation(out_ap, in_ap, AF.Sigmoid, bias=bias_ap)
                nc.vector.tensor_scalar_add(zscratch, in_ap, bias_ap)
                nc.vector.tensor_mul(out_ap, out_ap, zscratch)

        with ExitStack() as ctx:
            const = ctx.enter_context(tc.tile_pool(name="const", bufs=1))
            small = ctx.enter_context(tc.tile_pool(name="small", bufs=1))
            xpool = ctx.enter_context(tc.tile_pool(name="xp", bufs=1))
            xtp = ctx.enter_context(tc.tile_pool(name="xtp", bufs=2))
            sop = ctx.enter_context(tc.tile_pool(name="sop", bufs=2))
            zpool = ctx.enter_context(tc.tile_pool(name="zp", bufs=2))

            # ---- constant loads (small first so the head can start) ----
            def cload(name, shape):
                t = const.tile(shape, FP, tag=name, name=name)
                nc.sync.dma_start(out=t[:], in_=d[name][:])
                return t

            embT_t = cload("embT", [128, 2 * B_LOC])
            A_t = cload("A", [128, 256])
            w1T_t = cload("w1T", [128, 128])
            b1_t = cload("b1", [64, 1])
            w2T_t = cload("w2T", [64, 128])
            b2r_t = cload("b2r", [4, 128])
            lbr_t = cload("lbr", [4, 64])
            lwT2_t = cload("lwT2", [128, 64])
            ones_t = cload("ones", [1, 64])
            id128_t = cload("id128", [128, 128])
            ids_t = cload("idstack", [128, 32])
            nids_t = cload("nidstack", [128, 32])

            # x tiles [128, 2048] x 8 and F quarters, interleaved
            xt = [[xpool.tile([128, 2048], FP, tag=f"x{t}{q}", name=f"x{t}{q}")
                   for q in range(4)] for t in range(2)]
            Fq = [const.tile([128, 2048], FP, tag=f"F{q}", name=f"F{q}") for q in range(4)]
            for q in range(4):
                for t in range(2):
                    nc.sync.dma_start(
                        out=xt[t][q][:],
                        in_=d["x4"][t * 128:(t + 1) * 128, q * 2048:(q + 1) * 2048])
                nc.sync.dma_start(out=Fq[q][:],
                                  in_=d["F"][:, q * 2048:(q + 1) * 2048])
            W_t = cload("W", [128, 8192])
            Gq = [const.tile([128, 2048], FP, tag=f"G{q}", name=f"G{q}") for q in range(4)]
            for q in range(4):
                nc.sync.dma_start(out=Gq[q][:],
                                  in_=d["G"][:, q * 2048:(q + 1) * 2048])

            # ---- head: phi, MLP, scaled time weights, folded bias ----
            phi_sb = small.tile([128, B_LOC], FP, tag="phi")
            phi4rep = small.tile([128, 16], FP, tag="phi4rep")
            gbT_sb = small.tile([4, 128], FP, tag="gbT")
            gbrows = small.tile([1, 256], FP, tag="gbrows")
            biasvec = small.tile([4, 64], FP, tag="biasvec")
            bt = [small.tile([128, 1], FP, tag=f"bt{t}", name=f"bt{t}") for t in range(2)]
            linwb2 = [small.tile([128, 128], FP, tag=f"lw{t}", name=f"lw{t}") for t in range(2)]
            tmp44 = small.tile([4, 64], FP, tag="tmp44")

            with tc.tile_pool(name="ps_head", bufs=1, space="PSUM") as ph:
                phiT_p = ph.tile([B_LOC, 128], FP, tag="phiT")
                for kc in range(2):
                    nc.tensor.matmul(phiT_p[:],
                                     lhsT=embT_t[:, kc * 4:(kc + 1) * 4],
                                     rhs=A_t[:, kc * 128:(kc + 1) * 128],
                                     start=(kc == 0), stop=(kc == 1))
                phiT_sb = small.tile([B_LOC, 128], FP, tag="phiT_sb")
                nc.vector.tensor_copy(phiT_sb[:], phiT_p[:])
                phi_p = ph.tile([128, B_LOC], FP, tag="phip")
                nc.tensor.transpose(phi_p[:], phiT_sb[:], id128_t[0:4, 0:4])
                nc.vector.tensor_copy(phi_sb[:], phi_p[:])
                # phi4rep[32r+m, dd*8+br*4+b] = phi[dd*64+br*32+m, b]
                for dd in range(2):
                    for br in range(2):
                        nc.gpsimd.dma_start(
                            out=phi4rep[0:32, dd * 8 + br * 4:dd * 8 + br * 4 + 4],
                            in_=phi_sb[dd * 64 + br * 32:dd * 64 + br * 32 + 32, :])
                for r in range(1, 4):
                    nc.gpsimd.dma_start(out=phi4rep[32 * r:32 * (r + 1), :],
                                        in_=phi4rep[0:32, :])

                h_p = ph.tile([HID, B_LOC], FP, tag="h")
                for kc in range(2):
                    nc.tensor.matmul(h_p[:],
                                     lhsT=w1T_t[:, kc * 64:(kc + 1) * 64],
                                     rhs=embT_t[:, kc * 4:(kc + 1) * 4],
                                     start=(kc == 0), stop=(kc == 1))
                h_sb = small.tile([HID, B_LOC], FP, tag="h_sb")
                hz = small.tile([HID, B_LOC], FP, tag="hz")
                act_silu(h_sb[:], h_p[:], b1_t[:, 0:1], hz[:])

                gbT_p = ph.tile([4, 128], FP, tag="gbTp")
                nc.tensor.matmul(gbT_p[:], lhsT=h_sb[:], rhs=w2T_t[:],
                                 start=True, stop=True)
                nc.vector.tensor_add(gbT_sb[:], gbT_p[:], b2r_t[:])

                # biasvec = gamma*lin_b + lin_b + beta
                nc.vector.tensor_mul(tmp44[:], gbT_sb[:, 0:64], lbr_t[:])
                nc.vector.tensor_add(tmp44[:], tmp44[:], lbr_t[:])
                nc.vector.tensor_add(biasvec[:], tmp44[:], gbT_sb[:, 64:128])
                for t in range(2):
                    for j in range(2):
                        nc.gpsimd.dma_start(
                            out=bt[t][j * 64:(j + 1) * 64, :],
                            in_=biasvec[2 * t + j:2 * t + j + 1, :])

                # gbrows[0, b*64+o] = gamma[b, o] (partition-0 gather)
                nc.gpsimd.dma_start(out=gbrows[:], in_=gbT_sb[:, 0:64])
                # linwb2[t] is block-diagonal [(j,c), (j,o)]:
                # diag block j = lin_w.T * (1 + gamma[2t+j]) -> the time
                # branch becomes one K=128 matmul per chunk
                for t in range(2):
                    rep_p = ph.tile([128, 64], FP, tag="rep")
                    for j in range(2):
                        b = 2 * t + j
                        nc.tensor.matmul(rep_p[j * 64:(j + 1) * 64, :],
                                         lhsT=ones_t[:],
                                         rhs=gbrows[0:1, b * 64:(b + 1) * 64],
                                         start=True, stop=True)
                    nc.vector.memset(linwb2[t][0:64, 64:128], 0.0)
                    nc.vector.memset(linwb2[t][64:128, 0:64], 0.0)
                    for j in range(2):
                        sl = slice(j * 64, (j + 1) * 64)
                        nc.vector.tensor_mul(linwb2[t][sl, sl], lwT2_t[sl, :],
                                             rep_p[sl, :])
                        nc.vector.tensor_add(linwb2[t][sl, sl],
                                             linwb2[t][sl, sl], lwT2_t[sl, :])

            # ---- fwd DFT: RT[modecol, rows] ----
            RT_sb = small.tile([128, ROWS], FP, tag="RT")
            with tc.tile_pool(name="ps_fwd", bufs=3, space="PSUM") as pf, \
                 tc.tile_pool(name="ps_rt", bufs=1, space="PSUM") as prt:
                rtp = prt.tile([128, ROWS], FP, tag="rtp")
                for c in range(64):
                    q, kk = divmod(c, 16)
                    off = kk * 128
                    tp = pf.tile([128, 256], FP, tag="tp")
                    nc.tensor.transpose(tp[:, 0:128],
                                        xt[0][q][:, off:off + 128], id128_t[:])
                    nc.tensor.transpose(tp[:, 128:256],
                                        xt[1][q][:, off:off + 128], id128_t[:])
                    xts = xtp.tile([128, 256], FP, tag="xts")
                    nc.vector.tensor_copy(xts[:], tp[:])
                    nc.tensor.matmul(rtp[:], lhsT=Fq[q][:, off:off + 128],
                                     rhs=xts[:], start=(c == 0), stop=(c == 63))
                nc.vector.tensor_copy(RT_sb[:], rtp[:])

            # ---- XS with phi folded via diagonal transpose-matmuls ----
            # dtile quadrant (br,din) lives at partition base br*64+din*32;
            # slot (b, dout): din=0 -> {pr, pi}, din=1 -> {-pi, pr}
            dtile = small.tile([128, 256], FP, tag="dtile")
            for br in range(2):
                for din in range(2):
                    base = br * 64 + din * 32
                    psl = slice(base, base + 32)
                    for b in range(B_LOC):
                        cpr, cpi = br * 4 + b, 8 + br * 4 + b
                        s0 = slice((b * 2) * 32, (b * 2) * 32 + 32)
                        s1 = slice((b * 2 + 1) * 32, (b * 2 + 1) * 32 + 32)
                        if din == 0:
                            nc.vector.tensor_scalar_mul(
                                dtile[psl, s0], ids_t[psl, :],
                                phi4rep[psl, cpr:cpr + 1])
                            nc.vector.tensor_scalar_mul(
                                dtile[psl, s1], ids_t[psl, :],
                                phi4rep[psl, cpi:cpi + 1])
                        else:
                            nc.vector.tensor_scalar_mul(
                                dtile[psl, s0], nids_t[psl, :],
                                phi4rep[psl, cpi:cpi + 1])
                            nc.vector.tensor_scalar_mul(
                                dtile[psl, s1], ids_t[psl, :],
                                phi4rep[psl, cpr:cpr + 1])

            XS_sb = [small.tile([128, 128], FP, tag=f"XS{br}",
                                name=f"XS{br}") for br in range(2)]
            spec_sb = small.tile([64, 512], FP, tag="spec")
            R2f = small.tile([128, ROWS], FP, tag="R2f")
            with tc.tile_pool(name="ps_mid", bufs=1, space="PSUM") as pm:
                for br in range(2):
                    xsp = pm.tile([128, 128], FP, tag=f"xsp{br}",
                                  name=f"xsp{br}")
                    # regular matmul against the [64, 32] stacked-diagonal
                    # rhs: contracts over (din, m) partitions, summing the
                    # re/im contributions with phi folded in
                    for b in range(B_LOC):
                        psl = slice(br * 64, br * 64 + 64)
                        for dout in range(2):
                            fsl = slice((b * 2 + dout) * 32,
                                        (b * 2 + dout) * 32 + 32)
                            nc.tensor.matmul(
                                xsp[dout * 64:(dout + 1) * 64, b::4],
                                lhsT=RT_sb[psl, b * 64:(b + 1) * 64],
                                rhs=dtile[psl, fsl],
                                start=True, stop=True)
                    nc.vector.tensor_copy(XS_sb[br][:], xsp[:])

                # spectral matmuls
                spp = pm.tile([64, 512], FP, tag="spp")
                for br in range(2):
                    for m in range(M):
                        for dout in range(2):
                            # col order (dout, br, m) matches R2T partition
                            # order so the R2 transpose input is spec[:, b::4]
                            col = ((dout * 2 + br) * 32 + m) * 4
                            wcol = ((br * 32 + m) * 2 + dout) * 64
                            nc.tensor.matmul(
                                spp[:, col:col + 4],
                                lhsT=W_t[:, wcol:wcol + 64],
                                rhs=XS_sb[br][:, m * 4:(m + 1) * 4],
                                start=True, stop=True)
                nc.vector.tensor_copy(spec_sb[:], spp[:])

                # R2 transposes -> R2f [(d,br,m), (b,o)]
                r2p = pm.tile([128, ROWS], FP, tag="r2p")
                for b in range(B_LOC):
                    nc.tensor.transpose(r2p[:, b * 64:(b + 1) * 64],
                                        spec_sb[:, b::4], id128_t[0:64, 0:64])
                nc.vector.tensor_copy(R2f[:], r2p[:])

            # ---- inverse DFT + time branch + silu + store ----
            with tc.tile_pool(name="ps_out", bufs=6, space="PSUM") as po:
                for t in range(2):
                    for q in range(4):
                        pos = [po.tile([128, 512], FP, tag="po", name=f"po{t}{q}{_k}")
                               for _k in range(4)]
                        for kk in range(4):
                            nc.tensor.matmul(
                                pos[kk][:],
                                lhsT=R2f[:, t * 128:(t + 1) * 128],
                                rhs=Gq[q][:, kk * 512:(kk + 1) * 512],
                                start=True, stop=False)
                        for kk in range(4):
                            nc.tensor.matmul(
                                pos[kk][:],
                                lhsT=linwb2[t][:],
                                rhs=xt[t][q][:, kk * 512:(kk + 1) * 512],
                                start=False, stop=True)
                        for half in range(2):
                            so = sop.tile([128, 1024], FP, tag="so")
                            for kk2 in range(2):
                                kk = half * 2 + kk2
                                zs = (zpool.tile([128, 512], FP, tag="zs", name="zs")[:]
                                      if sim_safe else None)
                                act_silu(so[:, kk2 * 512:(kk2 + 1) * 512],
                                         pos[kk][:], bt[t][:, 0:1], zs)
                            nc.sync.dma_start(
                                out=y[t * 128:(t + 1) * 128,
                                      q * 2048 + half * 1024:
                                      q * 2048 + (half + 1) * 1024],
                                in_=so[:])

    if split:
        _split_multiwait(nc)
    return nc


_NC = None


def _get_nc():
    global _NC
    if _NC is None:
        _NC = _build_nc()
    return _NC


def kernel(**inputs):
    inputs = {k: np.asarray(v) for k, v in inputs.items()}
    x, emb = inputs["x"], inputs["emb"]
    consts = _build_constants(**{k: v for k, v in inputs.items()
                                 if k not in ("x", "emb")})
    nc = _get_nc()

    in_maps = []
    for core in range(N_CORES):
        b0 = core * B_LOC
        m = dict(consts)
        m["x4"] = np.ascontiguousarray(
            x[b0:b0 + B_LOC].reshape(ROWS, L), np.float32)
        eT = emb[b0:b0 + B_LOC].T.astype(np.float32)
        m["embT"] = np.ascontiguousarray(eT.reshape(2, 128, B_LOC).transpose(1, 0, 2).reshape(128, 2 * B_LOC))
        in_maps.append(m)

    res = run_bass_kernel_spmd(nc, in_maps, core_ids=list(range(N_CORES)))
    out = np.empty((B, C, L), np.float32)
    for core in range(N_CORES):
        b0 = core * B_LOC
        out[b0:b0 + B_LOC] = res.results[core]["y"].reshape(B_LOC, C, L)
    return out
